# revision 3
# baseline (speedup 1.0000x reference)
"""Trainium2 Bass kernel for nn_CortexBlock_59940563583556.

Math note (exact, not an approximation): the reference initializes the
fast-weight state U0 = V0 = 0 inside reference() itself, and every term
of the scan's update to U/V is proportional to ku = k_t^T @ U (zero when
U == 0).  By induction U_t == V_t == 0 for the whole scan, for ANY input
values.  Hence k_fast == 0, score_fast == 0, and (since mix_logit is
added to both logits, softmax is shift-invariant) the block reduces
exactly to:

    q = h @ Wq.T ; k = h @ Wk.T ; v = h @ Wv.T          (per-head split)
    g[b,t,h]  = sigmoid( sum_d q[b,t,h,d] * k[b,t,h,d] / sqrt(64) )
    out       = (g * v  per head) @ Wo.T

m_gate / alpha_scale / Wa / ba / mix_logit do not affect the output.

Sharding: the recurrence is gone, so we data-parallel the 8192 rows of
the flattened [B*T, D] activations across the 8 NeuronCores (1024 rows
each) and replicate the four 1024x1024 weight matrices.

Per-core dataflow (all compute on device):
  - weights DMA'd in fp32, cast to bf16 (GpSimd), DMA-transposed to
    W^T layout [128, 8, 1024] (d on partitions) -- one-time prep.
  - per 128-row tile: h cast to bf16 + DMA-transposed; q/k/v via PE
    matmuls (bf16, fp32 PSUM); s = per-head rowsum(q*k) on DVE;
    g = sigmoid(s/8) on ACT; y = g*v on DVE (bf16); y DMA-transposed;
    out = y @ Wo.T via PE; PSUM->SBUF copy on ACT; DMA out.
"""

import numpy as np

import concourse.bass as bass
import concourse.mybir as mybir
import concourse.tile as tile
from concourse import bacc
from concourse.bass_utils import run_bass_kernel_spmd

F32 = mybir.dt.float32
BF16 = mybir.dt.bfloat16

N_CORES = 8
D = 1024          # model dim
ROWS = 8192       # B*T
M_CORE = ROWS // N_CORES   # rows per core
P = 128           # partitions
KT = D // P       # contraction tiles
MT = M_CORE // P  # row tiles per core
NCH = 2           # output-column chunks of 512
CHW = D // NCH    # 512
H = 16            # heads
DH = 64           # head dim
INV_SQRT_DH = 1.0 / (DH ** 0.5)

_COMPILED = None  # (nc,) cache
LAST_RESULT = None  # BassKernelResults of the most recent run (for test harness)


def _build():
    nc = bacc.Bacc("TRN2", target_bir_lowering=False, debug=False)

    h_in = nc.dram_tensor("h", [M_CORE, D], F32, kind="ExternalInput")
    w_in = {
        name: nc.dram_tensor(name, [D, D], F32, kind="ExternalInput")
        for name in ("wq", "wk", "wv", "wo")
    }
    out = nc.dram_tensor("out", [M_CORE, D], F32, kind="ExternalOutput")

    with tile.TileContext(nc) as tc:
        with (
            tc.tile_pool(name="wt", bufs=1) as wt_pool,
            tc.tile_pool(name="wstage", bufs=3) as wstage_pool,
            tc.tile_pool(name="wbf", bufs=3) as wbf_pool,
            tc.tile_pool(name="hstage", bufs=2) as hstage_pool,
            tc.tile_pool(name="hbf", bufs=2) as hbf_pool,
            tc.tile_pool(name="hT", bufs=2) as hT_pool,
            tc.tile_pool(name="sp", bufs=2) as sp_pool,
            tc.tile_pool(name="small", bufs=4) as small_pool,
            tc.tile_pool(name="y", bufs=2) as y_pool,
            tc.tile_pool(name="yT", bufs=2) as yT_pool,
            tc.tile_pool(name="osb", bufs=2) as o_pool,
            tc.tile_pool(name="qk_ps", bufs=4, space="PSUM") as qk_psum,
            tc.tile_pool(name="v_ps", bufs=2, space="PSUM") as v_psum,
            tc.tile_pool(name="o_ps", bufs=2, space="PSUM") as o_psum,
        ):
            # ---- one-time: weights -> bf16, transposed, resident ----
            wT = {}
            for name in ("wq", "wk", "wv", "wo"):
                wT[name] = wt_pool.tile([P, KT, D], BF16, tag=f"wt_{name}", name=f"wt_{name}")
                for c in range(KT):
                    ws = wstage_pool.tile([P, D], F32, tag="ws")
                    nc.sync.dma_start(out=ws, in_=w_in[name][c * P:(c + 1) * P, :])
                    wb = wbf_pool.tile([P, D], BF16, tag="wb")
                    nc.gpsimd.tensor_copy(out=wb, in_=ws)
                    nc.sync.dma_start_transpose(
                        out=wT[name][:, :, c * P:(c + 1) * P], in_=wb
                    )

            # ---- per 128-row tile ----
            for i in range(MT):
                rows = slice(i * P, (i + 1) * P)
                hs = hstage_pool.tile([P, D], F32, tag="hs")
                nc.sync.dma_start(out=hs, in_=h_in[rows, :])
                hb = hbf_pool.tile([P, D], BF16, tag="hb")
                nc.vector.tensor_copy(out=hb, in_=hs)
                hT = hT_pool.tile([P, KT, P], BF16, tag="hT")
                nc.sync.dma_start_transpose(out=hT, in_=hb)

                # projections: q, k, v  (PSUM fp32, bf16 operands)
                q_ps, k_ps, v_ps = [], [], []
                for jo in range(NCH):
                    qp = qk_psum.tile([P, CHW], F32, tag="qk")
                    kp = qk_psum.tile([P, CHW], F32, tag="qk")
                    vp = v_psum.tile([P, CHW], F32, tag="v")
                    for (ps_t, wname) in ((qp, "wq"), (kp, "wk"), (vp, "wv")):
                        for kt in range(KT):
                            nc.tensor.matmul(
                                out=ps_t,
                                lhsT=hT[:, kt, :],
                                rhs=wT[wname][:, kt, jo * CHW:(jo + 1) * CHW],
                                start=(kt == 0),
                                stop=(kt == KT - 1),
                            )
                    q_ps.append(qp)
                    k_ps.append(kp)
                    v_ps.append(vp)

                # s[m, h] = sum_{d in head} q*k ; g = sigmoid(s/8)
                # (DVE can read only one PSUM operand: stage q in SBUF first)
                sp = sp_pool.tile([P, D], F32, tag="sp")
                for jo in range(NCH):
                    qsb = sp_pool.tile([P, CHW], BF16, tag="qsb")
                    nc.scalar.copy(out=qsb, in_=q_ps[jo])
                    nc.vector.tensor_mul(
                        out=sp[:, jo * CHW:(jo + 1) * CHW],
                        in0=qsb,
                        in1=k_ps[jo],
                    )
                s = small_pool.tile([P, H], F32, tag="s")
                nc.vector.reduce_sum(
                    out=s,
                    in_=sp.rearrange("p (h d) -> p h d", d=DH),
                    axis=mybir.AxisListType.X,
                )
                g = small_pool.tile([P, H], F32, tag="g")
                nc.scalar.activation(
                    out=g, in_=s,
                    func=mybir.ActivationFunctionType.Sigmoid,
                    scale=INV_SQRT_DH,
                )

                # y = g (broadcast over head dim) * v, in bf16
                y = y_pool.tile([P, D], BF16, tag="y")
                for jo in range(NCH):
                    g_sl = g[:, jo * (H // NCH):(jo + 1) * (H // NCH)]
                    g_bc = bass.AP(
                        tensor=g_sl.tensor, offset=g_sl.offset,
                        ap=[*g_sl.ap, [0, DH]],
                    )
                    nc.vector.tensor_mul(
                        out=y[:, jo * CHW:(jo + 1) * CHW].rearrange(
                            "p (h d) -> p h d", d=DH),
                        in0=v_ps[jo].rearrange("p (h d) -> p h d", d=DH),
                        in1=g_bc,
                    )

                yT = yT_pool.tile([P, KT, P], BF16, tag="yT")
                nc.sync.dma_start_transpose(out=yT, in_=y)

                # out = y @ Wo.T
                osb = o_pool.tile([P, D], F32, tag="osb")
                for jo in range(NCH):
                    op = o_psum.tile([P, CHW], F32, tag="o")
                    for kt in range(KT):
                        nc.tensor.matmul(
                            out=op,
                            lhsT=yT[:, kt, :],
                            rhs=wT["wo"][:, kt, jo * CHW:(jo + 1) * CHW],
                            start=(kt == 0),
                            stop=(kt == KT - 1),
                        )
                    nc.scalar.copy(out=osb[:, jo * CHW:(jo + 1) * CHW], in_=op)
                nc.sync.dma_start(out=out[rows, :], in_=osb)

    nc.compile()
    return nc


def kernel(hidden_states, m_gate, alpha_scale, Wq, Wk, Wv, Wo, Wa, ba, mix_logit,
           **_unused):
    global _COMPILED, LAST_RESULT
    if _COMPILED is None:
        _COMPILED = _build()
    nc = _COMPILED

    h = np.ascontiguousarray(
        np.asarray(hidden_states, dtype=np.float32).reshape(ROWS, D))
    wq = np.ascontiguousarray(np.asarray(Wq, dtype=np.float32))
    wk = np.ascontiguousarray(np.asarray(Wk, dtype=np.float32))
    wv = np.ascontiguousarray(np.asarray(Wv, dtype=np.float32))
    wo = np.ascontiguousarray(np.asarray(Wo, dtype=np.float32))

    in_maps = [
        {
            "h": np.ascontiguousarray(h[c * M_CORE:(c + 1) * M_CORE]),
            "wq": wq, "wk": wk, "wv": wv, "wo": wo,
        }
        for c in range(N_CORES)
    ]
    res = run_bass_kernel_spmd(nc, in_maps, core_ids=list(range(N_CORES)))
    LAST_RESULT = res
    out = np.concatenate([res.results[c]["out"] for c in range(N_CORES)], axis=0)
    B, T = 4, 2048
    return out.reshape(B, T, D)


# revision 5
# speedup vs baseline: 1.0491x; 1.0491x over previous
"""Trainium2 Bass kernel for nn_CortexBlock_59940563583556.

Math note (exact, not an approximation): the reference initializes the
fast-weight state U0 = V0 = 0 inside reference() itself, and every term
of the scan's update to U/V is proportional to ku = k_t^T @ U (zero when
U == 0).  By induction U_t == V_t == 0 for the whole scan, for ANY input
values.  Hence k_fast == 0, score_fast == 0, and (since mix_logit is
added to both logits, softmax is shift-invariant) the block reduces
exactly to:

    q = h @ Wq.T ; k = h @ Wk.T ; v = h @ Wv.T          (per-head split)
    g[b,t,h]  = sigmoid( sum_d q[b,t,h,d] * k[b,t,h,d] / sqrt(64) )
    out       = (g * v  per head) @ Wo.T

m_gate / alpha_scale / Wa / ba / mix_logit do not affect the output.

Sharding: the recurrence is gone, so we data-parallel the 8192 rows of
the flattened [B*T, D] activations across the 8 NeuronCores (1024 rows
each) and replicate the four 1024x1024 weight matrices.

Per-core dataflow (all compute on device):
  - weights DMA'd in fp32, cast to bf16 (GpSimd), DMA-transposed to
    W^T layout [128, 8, 1024] (d on partitions) -- one-time prep.
  - per 128-row tile: h cast to bf16 + DMA-transposed; q/k/v via PE
    matmuls (bf16, fp32 PSUM); s = per-head rowsum(q*k) on DVE;
    g = sigmoid(s/8) on ACT; y = g*v on DVE (bf16); y DMA-transposed;
    out = y @ Wo.T via PE; PSUM->SBUF copy on ACT; DMA out.
"""

import numpy as np

import concourse.bass as bass
import concourse.mybir as mybir
import concourse.tile as tile
from concourse import bacc
from concourse.bass_utils import run_bass_kernel_spmd

F32 = mybir.dt.float32
BF16 = mybir.dt.bfloat16

N_CORES = 8
D = 1024          # model dim
ROWS = 8192       # B*T
M_CORE = ROWS // N_CORES   # rows per core
P = 128           # partitions
KT = D // P       # contraction tiles
MT = M_CORE // P  # row tiles per core
NCH = 2           # output-column chunks of 512
CHW = D // NCH    # 512
H = 16            # heads
DH = 64           # head dim
INV_SQRT_DH = 1.0 / (DH ** 0.5)

_COMPILED = None  # (nc,) cache
LAST_RESULT = None  # BassKernelResults of the most recent run (for test harness)


def _build():
    nc = bacc.Bacc("TRN2", target_bir_lowering=False, debug=False)

    h_in = nc.dram_tensor("h", [M_CORE, D], F32, kind="ExternalInput")
    w_in = {
        name: nc.dram_tensor(name, [D, D], F32, kind="ExternalInput")
        for name in ("wq", "wk", "wv", "wo")
    }
    out = nc.dram_tensor("out", [M_CORE, D], F32, kind="ExternalOutput")

    with tile.TileContext(nc) as tc:
        with (
            tc.tile_pool(name="wt", bufs=1) as wt_pool,
            tc.tile_pool(name="wstage", bufs=3) as wstage_pool,
            tc.tile_pool(name="wbf", bufs=3) as wbf_pool,
            tc.tile_pool(name="hstage", bufs=2) as hstage_pool,
            tc.tile_pool(name="hbf", bufs=2) as hbf_pool,
            tc.tile_pool(name="hT", bufs=2) as hT_pool,
            tc.tile_pool(name="sp", bufs=2) as sp_pool,
            tc.tile_pool(name="small", bufs=4) as small_pool,
            tc.tile_pool(name="y", bufs=2) as y_pool,
            tc.tile_pool(name="yT", bufs=2) as yT_pool,
            tc.tile_pool(name="osb", bufs=2) as o_pool,
            tc.tile_pool(name="qk_ps", bufs=4, space="PSUM") as qk_psum,
            tc.tile_pool(name="v_ps", bufs=2, space="PSUM") as v_psum,
            tc.tile_pool(name="o_ps", bufs=2, space="PSUM") as o_psum,
        ):
            # ---- one-time: weights -> bf16, transposed, resident ----
            wT = {}
            for wi, name in enumerate(("wq", "wk", "wv", "wo")):
                wT[name] = wt_pool.tile([P, KT, D], BF16, tag=f"wt_{name}", name=f"wt_{name}")
                for c in range(KT):
                    ws = wstage_pool.tile([P, D], F32, tag="ws")
                    nc.sync.dma_start(out=ws, in_=w_in[name][c * P:(c + 1) * P, :])
                    wb = wbf_pool.tile([P, D], BF16, tag="wb")
                    # split casts between DVE and ACT (gpsimd is ~7x slower
                    # here and a single engine serializes the weight prep)
                    if (wi * KT + c) % 2 == 0:
                        nc.vector.tensor_copy(out=wb, in_=ws)
                    else:
                        nc.scalar.copy(out=wb, in_=ws)
                    nc.sync.dma_start_transpose(
                        out=wT[name][:, :, c * P:(c + 1) * P], in_=wb
                    )

            # ---- per 128-row tile ----
            for i in range(MT):
                rows = slice(i * P, (i + 1) * P)
                hs = hstage_pool.tile([P, D], F32, tag="hs")
                nc.sync.dma_start(out=hs, in_=h_in[rows, :])
                hb = hbf_pool.tile([P, D], BF16, tag="hb")
                nc.vector.tensor_copy(out=hb, in_=hs)
                hT = hT_pool.tile([P, KT, P], BF16, tag="hT")
                nc.sync.dma_start_transpose(out=hT, in_=hb)

                # projections: q, k, v  (PSUM fp32, bf16 operands)
                q_ps, k_ps, v_ps = [], [], []
                for jo in range(NCH):
                    qp = qk_psum.tile([P, CHW], F32, tag="qk")
                    kp = qk_psum.tile([P, CHW], F32, tag="qk")
                    vp = v_psum.tile([P, CHW], F32, tag="v")
                    for (ps_t, wname) in ((qp, "wq"), (kp, "wk"), (vp, "wv")):
                        for kt in range(KT):
                            nc.tensor.matmul(
                                out=ps_t,
                                lhsT=hT[:, kt, :],
                                rhs=wT[wname][:, kt, jo * CHW:(jo + 1) * CHW],
                                start=(kt == 0),
                                stop=(kt == KT - 1),
                            )
                    q_ps.append(qp)
                    k_ps.append(kp)
                    v_ps.append(vp)

                # s[m, h] = sum_{d in head} q*k ; g = sigmoid(s/8)
                # (DVE can read only one PSUM operand: stage q in SBUF first)
                sp = sp_pool.tile([P, D], F32, tag="sp")
                for jo in range(NCH):
                    qsb = sp_pool.tile([P, CHW], BF16, tag="qsb")
                    nc.scalar.copy(out=qsb, in_=q_ps[jo])
                    nc.vector.tensor_mul(
                        out=sp[:, jo * CHW:(jo + 1) * CHW],
                        in0=qsb,
                        in1=k_ps[jo],
                    )
                s = small_pool.tile([P, H], F32, tag="s")
                nc.vector.reduce_sum(
                    out=s,
                    in_=sp.rearrange("p (h d) -> p h d", d=DH),
                    axis=mybir.AxisListType.X,
                )
                g = small_pool.tile([P, H], F32, tag="g")
                nc.scalar.activation(
                    out=g, in_=s,
                    func=mybir.ActivationFunctionType.Sigmoid,
                    scale=INV_SQRT_DH,
                )

                # y = g (broadcast over head dim) * v, in bf16
                y = y_pool.tile([P, D], BF16, tag="y")
                for jo in range(NCH):
                    g_sl = g[:, jo * (H // NCH):(jo + 1) * (H // NCH)]
                    g_bc = bass.AP(
                        tensor=g_sl.tensor, offset=g_sl.offset,
                        ap=[*g_sl.ap, [0, DH]],
                    )
                    nc.vector.tensor_mul(
                        out=y[:, jo * CHW:(jo + 1) * CHW].rearrange(
                            "p (h d) -> p h d", d=DH),
                        in0=v_ps[jo].rearrange("p (h d) -> p h d", d=DH),
                        in1=g_bc,
                    )

                yT = yT_pool.tile([P, KT, P], BF16, tag="yT")
                nc.sync.dma_start_transpose(out=yT, in_=y)

                # out = y @ Wo.T
                osb = o_pool.tile([P, D], F32, tag="osb")
                for jo in range(NCH):
                    op = o_psum.tile([P, CHW], F32, tag="o")
                    for kt in range(KT):
                        nc.tensor.matmul(
                            out=op,
                            lhsT=yT[:, kt, :],
                            rhs=wT["wo"][:, kt, jo * CHW:(jo + 1) * CHW],
                            start=(kt == 0),
                            stop=(kt == KT - 1),
                        )
                    nc.scalar.copy(out=osb[:, jo * CHW:(jo + 1) * CHW], in_=op)
                nc.sync.dma_start(out=out[rows, :], in_=osb)

    nc.compile()
    return nc


def kernel(hidden_states, m_gate, alpha_scale, Wq, Wk, Wv, Wo, Wa, ba, mix_logit,
           **_unused):
    global _COMPILED, LAST_RESULT
    if _COMPILED is None:
        _COMPILED = _build()
    nc = _COMPILED

    h = np.ascontiguousarray(
        np.asarray(hidden_states, dtype=np.float32).reshape(ROWS, D))
    wq = np.ascontiguousarray(np.asarray(Wq, dtype=np.float32))
    wk = np.ascontiguousarray(np.asarray(Wk, dtype=np.float32))
    wv = np.ascontiguousarray(np.asarray(Wv, dtype=np.float32))
    wo = np.ascontiguousarray(np.asarray(Wo, dtype=np.float32))

    in_maps = [
        {
            "h": np.ascontiguousarray(h[c * M_CORE:(c + 1) * M_CORE]),
            "wq": wq, "wk": wk, "wv": wv, "wo": wo,
        }
        for c in range(N_CORES)
    ]
    res = run_bass_kernel_spmd(nc, in_maps, core_ids=list(range(N_CORES)))
    LAST_RESULT = res
    out = np.concatenate([res.results[c]["out"] for c in range(N_CORES)], axis=0)
    B, T = 4, 2048
    return out.reshape(B, T, D)
